# revision 11
# baseline (speedup 1.0000x reference)
"""Trainium2 Bass kernel for BinaryAssociativeMemory (chunked binary linear attention).

Sharding: 8 cores = 2 batches x 4 head-groups (4 heads each).
Per core: q/k/v projections from pre-transposed x (host supplies bf16 hi/mid
split), k/v binarized from a 3-term bf16 split matmul (sign-exact vs fp32),
chunked attention with a PSUM-resident running k^T v state, and a partial
output projection over the core's 4 heads. Host sums the 4 head-group
partials per batch.

The per-block schedule is software-pipelined: block tb's dense projection
matmuls are interleaved with block tb-1's attention chunks so the PE never
sees a sparse window (HAM stays un-throttled) and the attention's DVE
dependencies resolve during the projection units.
"""

import math
import numpy as np
import ml_dtypes

import concourse.bacc as bacc
import concourse.mybir as mybir
import concourse.tile as tile
from concourse.bass_utils import run_bass_kernel_spmd

dt = mybir.dt
AF = mybir.ActivationFunctionType

B, T, D = 2, 4096, 2048
H, DH, C = 16, 128, 64
NCORES = 8
GPB = 4            # head-groups per batch
HPC = H // GPB     # heads per core = 4
JW = HPC * DH      # per-core projection width = 512
TB = 512           # timestep block
NTB = T // TB      # 8
ND = D // 128      # 16 contraction blocks
NCH = TB // C      # 8 chunks per block
NCHUNK = T // C    # 64 chunks total

_cached_nc = None


def _build(debug=False):
    nc = bacc.Bacc("TRN2", target_bir_lowering=False, debug=False,
                   enable_asserts=False, num_devices=NCORES)
    f32, bf16 = dt.float32, dt.bfloat16

    XTH = nc.dram_tensor("XTH", [D, T], bf16, kind="ExternalInput").ap()
    XTM = nc.dram_tensor("XTM", [D, T], bf16, kind="ExternalInput").ap()
    WQTH = nc.dram_tensor("WQTH", [D, JW], bf16, kind="ExternalInput").ap()
    WKTH = nc.dram_tensor("WKTH", [D, JW], bf16, kind="ExternalInput").ap()
    WKTM = nc.dram_tensor("WKTM", [D, JW], bf16, kind="ExternalInput").ap()
    WVTH = nc.dram_tensor("WVTH", [D, JW], bf16, kind="ExternalInput").ap()
    WVTM = nc.dram_tensor("WVTM", [D, JW], bf16, kind="ExternalInput").ap()
    WOT = nc.dram_tensor("WOT", [JW, D], bf16, kind="ExternalInput").ap()
    MSK = nc.dram_tensor("MSK", [128, HPC * C], f32, kind="ExternalInput").ap()
    IDN = nc.dram_tensor("IDN", [128, 128], bf16, kind="ExternalInput").ap()
    OUT = nc.dram_tensor("OUT", [T, D], f32, kind="ExternalOutput").ap()
    OST = nc.dram_tensor("OST", [128, JW], f32, kind="ExternalOutput").ap()
    if debug:
        QDBG = nc.dram_tensor("QDBG", [JW, T], bf16, kind="ExternalOutput").ap()
        KDBG = nc.dram_tensor("KDBG", [JW, T], bf16, kind="ExternalOutput").ap()
        VDBG = nc.dram_tensor("VDBG", [T, JW], bf16, kind="ExternalOutput").ap()
        KKDBG = nc.dram_tensor("KKDBG", [T, JW], bf16, kind="ExternalOutput").ap()
        ATDBG = nc.dram_tensor("ATDBG", [JW, T], bf16, kind="ExternalOutput").ap()

    with tile.TileContext(nc) as tc:
        with (
            tc.tile_pool(name="const", bufs=1) as cst,
            tc.tile_pool(name="xblk", bufs=1) as xblk,
            tc.tile_pool(name="proj", bufs=8) as projp,
            tc.tile_pool(name="small", bufs=4) as smallp,
            tc.tile_pool(name="pfx", bufs=2) as pfxp,
            tc.tile_pool(name="bigps", bufs=4, space="PSUM") as bigps,
            tc.tile_pool(name="stps", bufs=1, space="PSUM") as stps,
            tc.tile_pool(name="scat", bufs=3, space="PSUM") as scat,
        ):
            # ---- constants / weights resident in SBUF ----
            def load_w(name, dram):
                ts = []
                for kb in range(ND):
                    t = cst.tile([128, JW], bf16, tag=f"{name}{kb}", name=f"{name}{kb}")
                    nc.sync.dma_start(t[:], dram[kb * 128:(kb + 1) * 128, :])
                    ts.append(t)
                return ts

            wq = load_w("wq", WQTH)
            wo = cst.tile([128, HPC * D], bf16, tag="wo")
            for hh in range(HPC):
                nc.sync.dma_start(wo[:, hh * D:(hh + 1) * D],
                                  WOT[hh * 128:(hh + 1) * 128, :])
            msk = cst.tile([128, HPC * C], f32, tag="msk")
            nc.sync.dma_start(msk[:], MSK[:])
            idn = cst.tile([128, 128], bf16, tag="idn")
            nc.sync.dma_start(idn[:], IDN[:])

            st_ps = stps.tile([128, JW], f32, tag="st")
            pipe = {"pf": None}
            wdefer = {}

            # ---------- emission units ----------
            def emit_dma_h(tb, blk):
                t0 = tb * TB
                xh = xblk.tile([128, ND * TB], bf16, tag="xh", name=f"xh{tb}", bufs=2)
                for kb in range(ND):
                    nc.sync.dma_start(xh[:, kb * TB:(kb + 1) * TB],
                                      XTH[kb * 128:(kb + 1) * 128, t0:t0 + TB])
                blk["xh"] = xh

            def emit_dma_m(tb, blk):
                t0 = tb * TB
                xm = xblk.tile([128, ND * TB], bf16, tag="xm", name=f"xm{tb}", bufs=1)
                for kb in range(ND):
                    nc.sync.dma_start(xm[:, kb * TB:(kb + 1) * TB],
                                      XTM[kb * 128:(kb + 1) * 128, t0:t0 + TB])
                blk["xm"] = xm

            def emit_q(tb, blk, hh):
                ps = bigps.tile([128, TB], f32, tag="bigps", name=f"psq{tb}_{hh}")
                for kb in range(ND):
                    nc.tensor.matmul(
                        ps[:], wq[kb][:, hh * 128:(hh + 1) * 128],
                        blk["xh"][:, kb * TB:(kb + 1) * TB],
                        start=(kb == 0), stop=(kb == ND - 1))
                q_h = projp.tile([128, TB], bf16, tag="qt", name=f"qt{tb}_{hh}")
                nc.vector.tensor_copy(q_h[:], ps[:])
                if debug:
                    t0 = tb * TB
                    nc.sync.dma_start(QDBG[hh * 128:(hh + 1) * 128, t0:t0 + TB], q_h[:])
                blk["qt"].append(q_h)

            def emit_k(tb, blk, hh):
                ps = bigps.tile([128, TB], f32, tag="bigps", name=f"psk{tb}_{hh}")
                i, n_mm = 0, 3 * ND
                for kb in range(ND):
                    for w_t, x_t in ((wdefer["wkh"], blk["xh"]), (wdefer["wkh"], blk["xm"]), (wdefer["wkm"], blk["xh"])):
                        nc.tensor.matmul(
                            ps[:], w_t[kb][:, hh * 128:(hh + 1) * 128],
                            x_t[:, kb * TB:(kb + 1) * TB],
                            start=(i == 0), stop=(i == n_mm - 1))
                        i += 1
                k_h = projp.tile([128, TB], bf16, tag="kt", name=f"kt{tb}_{hh}")
                nc.scalar.activation(k_h[:], ps[:], AF.Sign)
                if debug:
                    t0 = tb * TB
                    nc.sync.dma_start(KDBG[hh * 128:(hh + 1) * 128, t0:t0 + TB], k_h[:])
                blk["kt"].append(k_h)

            def emit_v(tb, blk, ts):
                ps = bigps.tile([128, JW], f32, tag="bigps", name=f"psv{tb}_{ts}")
                i, n_mm = 0, 3 * ND
                for kb in range(ND):
                    for x_t, w_t in ((blk["xh"], wdefer["wvh"]), (blk["xh"], wdefer["wvm"]), (blk["xm"], wdefer["wvh"])):
                        nc.tensor.matmul(
                            ps[:], x_t[:, kb * TB + ts * 128: kb * TB + (ts + 1) * 128],
                            w_t[kb][:],
                            start=(i == 0), stop=(i == n_mm - 1))
                        i += 1
                v_ts = projp.tile([128, JW], bf16, tag="vv", name=f"vv{tb}_{ts}")
                nc.scalar.activation(v_ts[:], ps[:], AF.Sign)
                if debug:
                    t0 = tb * TB
                    nc.sync.dma_start(VDBG[t0 + ts * 128: t0 + (ts + 1) * 128, :], v_ts[:])
                blk["vv"].append(v_ts)

            def emit_ktr(tb, blk, ts):
                k_ts = projp.tile([128, JW], bf16, tag="kk", name=f"kk{tb}_{ts}")
                for hh in range(HPC):
                    tp = scat.tile([128, 128], bf16, tag="scat", name=f"tp{tb}_{ts}_{hh}")
                    nc.tensor.transpose(tp[:], blk["kt"][hh][:, ts * 128:(ts + 1) * 128],
                                        idn[:])
                    nc.vector.tensor_copy(k_ts[:, hh * 128:(hh + 1) * 128], tp[:])
                if debug:
                    t0 = tb * TB
                    nc.sync.dma_start(KKDBG[t0 + ts * 128: t0 + (ts + 1) * 128, :], k_ts[:])
                blk["kk"].append(k_ts)

            def emit_sc(tb, blk, ch):
                # scores for all 4 heads of one chunk -> one PSUM bank; mask in 1 DVE op
                ro = (ch % 2) * C
                c0 = ch * C
                sc = scat.tile([128, HPC * C], f32, tag="scat", name=f"sc{tb}_{ch}")
                for hh in range(HPC):
                    nc.tensor.matmul(sc[ro:ro + C, hh * C:(hh + 1) * C],
                                     blk["kt"][hh][:, c0:c0 + C],
                                     blk["qt"][hh][:, c0:c0 + C],
                                     start=(hh == 0), stop=True,
                                     skip_group_check=True)
                sT = smallp.tile([128, HPC * C], bf16, tag="sT", name=f"sT{tb}_{ch}")
                nc.vector.tensor_mul(sT[ro:ro + C, :], sc[ro:ro + C, :], msk[ro:ro + C, :])
                blk["sT"][ch] = sT

            def emit_rest(tb, blk, ch):
                gn = tb * NCH + ch
                ts, ro = ch // 2, (ch % 2) * C
                c0 = ch * C
                if not blk["at"]:
                    blk["at"] = [projp.tile([128, TB], bf16, tag="at", bufs=5,
                                            name=f"at{tb}_{hh}") for hh in range(HPC)]
                sT = blk["sT"].pop(ch)
                ap = scat.tile([128, HPC * C], f32, tag="scat", name=f"ap{tb}_{ch}")
                for hh in range(HPC):
                    nc.tensor.matmul(ap[:, hh * C:(hh + 1) * C],
                                     blk["vv"][ts][ro:ro + C, hh * 128:(hh + 1) * 128],
                                     sT[ro:ro + C, hh * C:(hh + 1) * C],
                                     start=(hh == 0), stop=(gn == 0),
                                     skip_group_check=True)
                if gn > 0:
                    for hh in range(HPC):
                        nc.tensor.matmul(ap[:, hh * C:(hh + 1) * C],
                                         pipe["pf"][:, hh * 128:(hh + 1) * 128],
                                         blk["qt"][hh][:, c0:c0 + C],
                                         start=False, stop=True,
                                         skip_group_check=True)
                for hh in range(HPC):
                    nc.vector.tensor_copy(blk["at"][hh][:, c0:c0 + C],
                                          ap[:, hh * C:(hh + 1) * C])
                for hh in range(HPC):
                    # start=True clears has_written for the WHOLE bank: only the
                    # very first write of the kernel-long group may set it.
                    nc.tensor.matmul(st_ps[:, hh * 128:(hh + 1) * 128],
                                     blk["kk"][ts][ro:ro + C, hh * 128:(hh + 1) * 128],
                                     blk["vv"][ts][ro:ro + C, hh * 128:(hh + 1) * 128],
                                     start=(gn == 0 and hh == 0),
                                     stop=(gn == NCHUNK - 1),
                                     skip_group_check=True)
                if gn < NCHUNK - 1:
                    pf_new = pfxp.tile([128, JW], bf16, tag="pf", name=f"pf{gn}")
                    nc.vector.tensor_copy(pf_new[:], st_ps[:])
                    pipe["pf"] = pf_new

            def emit_out(tb, blk, ts, fb):
                t0 = tb * TB
                op = bigps.tile([128, 512], f32, tag="bigps", name=f"po{tb}_{ts}_{fb}")
                for hh in range(HPC):
                    nc.tensor.matmul(
                        op[:], blk["at"][hh][:, ts * 128:(ts + 1) * 128],
                        wo[:, hh * D + fb * 512: hh * D + (fb + 1) * 512],
                        start=(hh == 0), stop=(hh == HPC - 1))
                ob = smallp.tile([128, 512], f32, tag="ob", bufs=3, name=f"ob{tb}_{ts}_{fb}")
                nc.vector.tensor_copy(ob[:], op[:])
                nc.sync.dma_start(
                    OUT[t0 + ts * 128: t0 + (ts + 1) * 128,
                        fb * 512:(fb + 1) * 512], ob[:])

            # ---------- pipelined schedule ----------
            def AU(prev_tb, prev, i):
                """attention micro-unit i (0..8) for the previous block"""
                if prev is None:
                    return
                if i < NCH:
                    emit_sc(prev_tb, prev, i)
                if i >= 1:
                    emit_rest(prev_tb, prev, i - 1)

            blocks = {}
            def mkblk():
                return {"qt": [], "kt": [], "vv": [], "kk": [], "at": None,
                        "sT": {}}

            blocks[0] = mkblk()
            emit_dma_h(0, blocks[0])
            wdefer["wkh"] = load_w("wkh", WKTH)
            wdefer["wkm"] = load_w("wkm", WKTM)
            emit_dma_m(0, blocks[0])
            wdefer["wvh"] = load_w("wvh", WVTH)
            wdefer["wvm"] = load_w("wvm", WVTM)

            for tb in range(NTB + 1):
                cur = blocks.get(tb)
                prev = blocks.get(tb - 1)
                ptb = tb - 1
                if cur is not None:
                    if tb + 1 < NTB:
                        blocks[tb + 1] = mkblk()
                        emit_dma_h(tb + 1, blocks[tb + 1])
                    for hh in range(HPC):
                        emit_q(tb, cur, hh)
                        AU(ptb, prev, hh)
                    for hh in range(HPC):
                        emit_k(tb, cur, hh)
                        AU(ptb, prev, HPC + hh)
                    emit_v(tb, cur, 0)
                    AU(ptb, prev, 8)
                    for ts in range(1, HPC):
                        emit_v(tb, cur, ts)
                    if tb + 1 < NTB:
                        emit_dma_m(tb + 1, blocks[tb + 1])
                    if prev is not None:
                        if debug:
                            for hh in range(HPC):
                                nc.sync.dma_start(
                                    ATDBG[hh * 128:(hh + 1) * 128,
                                          ptb * TB:(ptb + 1) * TB],
                                    prev["at"][hh][:])
                        for ts in range(HPC):
                            for fb in range(4):
                                emit_out(ptb, prev, ts, fb)
                    for ts in range(HPC):
                        emit_ktr(tb, cur, ts)
                else:
                    for i in range(NCH + 1):
                        AU(ptb, prev, i)
                    if debug:
                        for hh in range(HPC):
                            nc.sync.dma_start(
                                ATDBG[hh * 128:(hh + 1) * 128,
                                      ptb * TB:(ptb + 1) * TB],
                                prev["at"][hh][:])
                    for ts in range(HPC):
                        for fb in range(4):
                            emit_out(ptb, prev, ts, fb)
                blocks.pop(tb - 2, None)

            stf = cst.tile([128, JW], f32, tag="stf")
            nc.vector.tensor_copy(stf[:], st_ps[:])
            nc.sync.dma_start(OST[:], stf[:])

    nc.compile()
    return nc


def _get_nc():
    global _cached_nc
    if _cached_nc is None:
        _cached_nc = _build()
    return _cached_nc


def _prep_inputs(x, Wq, Wk, Wv, Wo):
    bf = ml_dtypes.bfloat16
    f32 = np.float32
    s = 1.0 / math.sqrt(DH)

    def split_t(W):
        Wh = W.astype(bf)
        Wm = (W - Wh.astype(f32)).astype(bf)
        return (np.ascontiguousarray(Wh.T), np.ascontiguousarray(Wm.T))

    xts = []
    for b in range(B):
        xb = np.asarray(x[b], dtype=f32)
        xh = xb.astype(bf)
        xm = (xb - xh.astype(f32)).astype(bf)
        xts.append((np.ascontiguousarray(xh.T), np.ascontiguousarray(xm.T)))

    m1 = np.triu(np.ones((C, C), dtype=f32))
    mask = np.ascontiguousarray(np.tile(np.vstack([m1, m1]), (1, HPC)))
    idn = np.eye(128, dtype=f32).astype(bf)

    in_maps = []
    for c in range(NCORES):
        b, g = divmod(c, GPB)
        rows = slice(g * JW, (g + 1) * JW)
        wqth = np.ascontiguousarray(((Wq[rows] * s).astype(bf)).T)
        wkth, wktm = split_t(Wk[rows])
        wvth, wvtm = split_t(Wv[rows])
        wot = np.ascontiguousarray(Wo[:, rows].T.astype(bf))
        in_maps.append({
            "XTH": xts[b][0], "XTM": xts[b][1],
            "WQTH": wqth, "WKTH": wkth, "WKTM": wktm,
            "WVTH": wvth, "WVTM": wvtm, "WOT": wot,
            "MSK": mask, "IDN": idn,
        })
    return in_maps


def kernel(x, Wq, Wk, Wv, Wo, _trace=False, _trace_kwargs=None):
    x, Wq, Wk, Wv, Wo = (np.asarray(a, dtype=np.float32)
                         for a in (x, Wq, Wk, Wv, Wo))
    nc = _get_nc()
    in_maps = _prep_inputs(x, Wq, Wk, Wv, Wo)
    res = run_bass_kernel_spmd(nc, in_maps, list(range(NCORES)),
                               trace=_trace, **(_trace_kwargs or {}))
    out = np.zeros((B, T, D), np.float32)
    state = np.zeros((B, H, DH, DH), np.float32)
    for c in range(NCORES):
        b, g = divmod(c, GPB)
        out[b] += res.results[c]["OUT"]
        st = res.results[c]["OST"]
        for hh in range(HPC):
            state[b, g * HPC + hh] = st[:, hh * 128:(hh + 1) * 128]
    kernel.last_result = res
    return out, state


# revision 12
# speedup vs baseline: 1.0142x; 1.0142x over previous
"""Trainium2 Bass kernel for BinaryAssociativeMemory (chunked binary linear attention).

Sharding: 8 cores = 2 batches x 4 head-groups (4 heads each).
Per core: q/k/v projections from pre-transposed x (host supplies bf16 hi/mid
split), k/v binarized from a 3-term bf16 split matmul (sign-exact vs fp32),
chunked attention with a PSUM-resident running k^T v state, and a partial
output projection over the core's 4 heads. Host sums the 4 head-group
partials per batch.

The per-block schedule is software-pipelined: block tb's dense projection
matmuls are interleaved with block tb-1's attention chunks so the PE never
sees a sparse window (HAM stays un-throttled) and the attention's DVE
dependencies resolve during the projection units.
"""

import math
import numpy as np
import ml_dtypes

import concourse.bacc as bacc
import concourse.mybir as mybir
import concourse.tile as tile
from concourse.bass_utils import run_bass_kernel_spmd

dt = mybir.dt
AF = mybir.ActivationFunctionType

B, T, D = 2, 4096, 2048
H, DH, C = 16, 128, 64
NCORES = 8
GPB = 4            # head-groups per batch
HPC = H // GPB     # heads per core = 4
JW = HPC * DH      # per-core projection width = 512
TB = 512           # timestep block
NTB = T // TB      # 8
ND = D // 128      # 16 contraction blocks
NCH = TB // C      # 8 chunks per block
NCHUNK = T // C    # 64 chunks total

_cached_nc = None


def _build(debug=False):
    nc = bacc.Bacc("TRN2", target_bir_lowering=False, debug=False,
                   enable_asserts=False, num_devices=NCORES)
    f32, bf16 = dt.float32, dt.bfloat16

    XTH = nc.dram_tensor("XTH", [D, T], bf16, kind="ExternalInput").ap()
    XTM = nc.dram_tensor("XTM", [D, T], bf16, kind="ExternalInput").ap()
    WQTH = nc.dram_tensor("WQTH", [D, JW], bf16, kind="ExternalInput").ap()
    WKTH = nc.dram_tensor("WKTH", [D, JW], bf16, kind="ExternalInput").ap()
    WKTM = nc.dram_tensor("WKTM", [D, JW], bf16, kind="ExternalInput").ap()
    WVTH = nc.dram_tensor("WVTH", [D, JW], bf16, kind="ExternalInput").ap()
    WVTM = nc.dram_tensor("WVTM", [D, JW], bf16, kind="ExternalInput").ap()
    WOT = nc.dram_tensor("WOT", [JW, D], bf16, kind="ExternalInput").ap()
    MSK = nc.dram_tensor("MSK", [128, HPC * C], f32, kind="ExternalInput").ap()
    IDN = nc.dram_tensor("IDN", [128, 128], bf16, kind="ExternalInput").ap()
    OUT = nc.dram_tensor("OUT", [T, D], f32, kind="ExternalOutput").ap()
    OST = nc.dram_tensor("OST", [128, JW], f32, kind="ExternalOutput").ap()
    if debug:
        QDBG = nc.dram_tensor("QDBG", [JW, T], bf16, kind="ExternalOutput").ap()
        KDBG = nc.dram_tensor("KDBG", [JW, T], bf16, kind="ExternalOutput").ap()
        VDBG = nc.dram_tensor("VDBG", [T, JW], bf16, kind="ExternalOutput").ap()
        KKDBG = nc.dram_tensor("KKDBG", [T, JW], bf16, kind="ExternalOutput").ap()
        ATDBG = nc.dram_tensor("ATDBG", [JW, T], bf16, kind="ExternalOutput").ap()

    with tile.TileContext(nc) as tc:
        with (
            tc.tile_pool(name="const", bufs=1) as cst,
            tc.tile_pool(name="xblk", bufs=1) as xblk,
            tc.tile_pool(name="proj", bufs=8) as projp,
            tc.tile_pool(name="small", bufs=4) as smallp,
            tc.tile_pool(name="pfx", bufs=2) as pfxp,
            tc.tile_pool(name="bigps", bufs=4, space="PSUM") as bigps,
            tc.tile_pool(name="stps", bufs=1, space="PSUM") as stps,
            tc.tile_pool(name="scat", bufs=3, space="PSUM") as scat,
        ):
            # ---- constants / weights resident in SBUF ----
            def load_w(name, dram):
                t = cst.tile([128, ND * JW], bf16, tag=name, name=name)
                for kb in range(ND):
                    nc.sync.dma_start(t[:, kb * JW:(kb + 1) * JW],
                                      dram[kb * 128:(kb + 1) * 128, :])
                return t

            wq = load_w("wq", WQTH)
            wo = cst.tile([128, HPC * D], bf16, tag="wo")
            for hh in range(HPC):
                nc.sync.dma_start(wo[:, hh * D:(hh + 1) * D],
                                  WOT[hh * 128:(hh + 1) * 128, :])
            msk = cst.tile([128, HPC * C], f32, tag="msk")
            nc.sync.dma_start(msk[:], MSK[:])
            idn = cst.tile([128, 128], bf16, tag="idn")
            nc.sync.dma_start(idn[:], IDN[:])

            st_ps = stps.tile([128, JW], f32, tag="st")
            pipe = {"pf": None}
            wdefer = {}

            # ---------- emission units ----------
            def emit_dma_h(tb, blk):
                t0 = tb * TB
                xh = xblk.tile([128, ND * TB], bf16, tag="xh", name=f"xh{tb}", bufs=2)
                for kb in range(ND):
                    nc.sync.dma_start(xh[:, kb * TB:(kb + 1) * TB],
                                      XTH[kb * 128:(kb + 1) * 128, t0:t0 + TB])
                blk["xh"] = xh

            def emit_dma_m(tb, blk):
                t0 = tb * TB
                xm = xblk.tile([128, ND * TB], bf16, tag="xm", name=f"xm{tb}", bufs=1)
                for kb in range(ND):
                    nc.sync.dma_start(xm[:, kb * TB:(kb + 1) * TB],
                                      XTM[kb * 128:(kb + 1) * 128, t0:t0 + TB])
                blk["xm"] = xm

            def emit_q(tb, blk, hh):
                ps = bigps.tile([128, TB], f32, tag="bigps", name=f"psq{tb}_{hh}")
                for kb in range(ND):
                    nc.tensor.matmul(
                        ps[:], wq[:, kb * JW + hh * 128: kb * JW + (hh + 1) * 128],
                        blk["xh"][:, kb * TB:(kb + 1) * TB],
                        start=(kb == 0), stop=(kb == ND - 1))
                q_h = projp.tile([128, TB], bf16, tag="qt", name=f"qt{tb}_{hh}")
                nc.vector.tensor_copy(q_h[:], ps[:])
                if debug:
                    t0 = tb * TB
                    nc.sync.dma_start(QDBG[hh * 128:(hh + 1) * 128, t0:t0 + TB], q_h[:])
                blk["qt"].append(q_h)

            def emit_k(tb, blk, hh):
                ps = bigps.tile([128, TB], f32, tag="bigps", name=f"psk{tb}_{hh}")
                i, n_mm = 0, 3 * ND
                for kb in range(ND):
                    for w_t, x_t in ((wdefer["wkh"], blk["xh"]), (wdefer["wkh"], blk["xm"]), (wdefer["wkm"], blk["xh"])):
                        nc.tensor.matmul(
                            ps[:], w_t[:, kb * JW + hh * 128: kb * JW + (hh + 1) * 128],
                            x_t[:, kb * TB:(kb + 1) * TB],
                            start=(i == 0), stop=(i == n_mm - 1))
                        i += 1
                k_h = projp.tile([128, TB], bf16, tag="kt", name=f"kt{tb}_{hh}")
                nc.scalar.activation(k_h[:], ps[:], AF.Sign)
                if debug:
                    t0 = tb * TB
                    nc.sync.dma_start(KDBG[hh * 128:(hh + 1) * 128, t0:t0 + TB], k_h[:])
                blk["kt"].append(k_h)

            def emit_v(tb, blk, ts):
                ps = bigps.tile([128, JW], f32, tag="bigps", name=f"psv{tb}_{ts}")
                i, n_mm = 0, 3 * ND
                for kb in range(ND):
                    for x_t, w_t in ((blk["xh"], wdefer["wvh"]), (blk["xh"], wdefer["wvm"]), (blk["xm"], wdefer["wvh"])):
                        nc.tensor.matmul(
                            ps[:], x_t[:, kb * TB + ts * 128: kb * TB + (ts + 1) * 128],
                            w_t[:, kb * JW:(kb + 1) * JW],
                            start=(i == 0), stop=(i == n_mm - 1))
                        i += 1
                v_ts = projp.tile([128, JW], bf16, tag="vv", name=f"vv{tb}_{ts}")
                nc.scalar.activation(v_ts[:], ps[:], AF.Sign)
                if debug:
                    t0 = tb * TB
                    nc.sync.dma_start(VDBG[t0 + ts * 128: t0 + (ts + 1) * 128, :], v_ts[:])
                blk["vv"].append(v_ts)

            def emit_ktr(tb, blk, ts):
                k_ts = projp.tile([128, JW], bf16, tag="kk", name=f"kk{tb}_{ts}")
                for hh in range(HPC):
                    tp = scat.tile([128, 128], bf16, tag="scat", name=f"tp{tb}_{ts}_{hh}")
                    nc.tensor.transpose(tp[:], blk["kt"][hh][:, ts * 128:(ts + 1) * 128],
                                        idn[:])
                    nc.vector.tensor_copy(k_ts[:, hh * 128:(hh + 1) * 128], tp[:])
                if debug:
                    t0 = tb * TB
                    nc.sync.dma_start(KKDBG[t0 + ts * 128: t0 + (ts + 1) * 128, :], k_ts[:])
                blk["kk"].append(k_ts)

            def emit_sc(tb, blk, ch):
                # scores for all 4 heads of one chunk -> one PSUM bank; mask in 1 DVE op
                ro = (ch % 2) * C
                c0 = ch * C
                sc = scat.tile([128, HPC * C], f32, tag="scat", name=f"sc{tb}_{ch}")
                for hh in range(HPC):
                    nc.tensor.matmul(sc[ro:ro + C, hh * C:(hh + 1) * C],
                                     blk["kt"][hh][:, c0:c0 + C],
                                     blk["qt"][hh][:, c0:c0 + C],
                                     start=(hh == 0), stop=True,
                                     skip_group_check=True)
                sT = smallp.tile([128, HPC * C], bf16, tag="sT", name=f"sT{tb}_{ch}")
                nc.vector.tensor_mul(sT[ro:ro + C, :], sc[ro:ro + C, :], msk[ro:ro + C, :])
                blk["sT"][ch] = sT

            def emit_rest(tb, blk, ch):
                gn = tb * NCH + ch
                ts, ro = ch // 2, (ch % 2) * C
                c0 = ch * C
                if not blk["at"]:
                    blk["at"] = [projp.tile([128, TB], bf16, tag="at", bufs=5,
                                            name=f"at{tb}_{hh}") for hh in range(HPC)]
                sT = blk["sT"].pop(ch)
                ap = scat.tile([128, HPC * C], f32, tag="scat", name=f"ap{tb}_{ch}")
                for hh in range(HPC):
                    nc.tensor.matmul(ap[:, hh * C:(hh + 1) * C],
                                     blk["vv"][ts][ro:ro + C, hh * 128:(hh + 1) * 128],
                                     sT[ro:ro + C, hh * C:(hh + 1) * C],
                                     start=(hh == 0), stop=(gn == 0),
                                     skip_group_check=True)
                if gn > 0:
                    for hh in range(HPC):
                        nc.tensor.matmul(ap[:, hh * C:(hh + 1) * C],
                                         pipe["pf"][:, hh * 128:(hh + 1) * 128],
                                         blk["qt"][hh][:, c0:c0 + C],
                                         start=False, stop=True,
                                         skip_group_check=True)
                for hh in range(HPC):
                    nc.vector.tensor_copy(blk["at"][hh][:, c0:c0 + C],
                                          ap[:, hh * C:(hh + 1) * C])
                for hh in range(HPC):
                    # start=True clears has_written for the WHOLE bank: only the
                    # very first write of the kernel-long group may set it.
                    nc.tensor.matmul(st_ps[:, hh * 128:(hh + 1) * 128],
                                     blk["kk"][ts][ro:ro + C, hh * 128:(hh + 1) * 128],
                                     blk["vv"][ts][ro:ro + C, hh * 128:(hh + 1) * 128],
                                     start=(gn == 0 and hh == 0),
                                     stop=(gn == NCHUNK - 1),
                                     skip_group_check=True)
                if gn < NCHUNK - 1:
                    pf_new = pfxp.tile([128, JW], bf16, tag="pf", name=f"pf{gn}")
                    nc.vector.tensor_copy(pf_new[:], st_ps[:])
                    pipe["pf"] = pf_new

            def emit_out(tb, blk, ts, fb):
                t0 = tb * TB
                op = bigps.tile([128, 512], f32, tag="bigps", name=f"po{tb}_{ts}_{fb}")
                for hh in range(HPC):
                    nc.tensor.matmul(
                        op[:], blk["at"][hh][:, ts * 128:(ts + 1) * 128],
                        wo[:, hh * D + fb * 512: hh * D + (fb + 1) * 512],
                        start=(hh == 0), stop=(hh == HPC - 1))
                ob = smallp.tile([128, 512], f32, tag="ob", bufs=3, name=f"ob{tb}_{ts}_{fb}")
                nc.any.tensor_copy(ob[:], op[:])
                nc.sync.dma_start(
                    OUT[t0 + ts * 128: t0 + (ts + 1) * 128,
                        fb * 512:(fb + 1) * 512], ob[:])

            # ---------- pipelined schedule ----------
            def AU(prev_tb, prev, i):
                """attention micro-unit i (0..8) for the previous block"""
                if prev is None:
                    return
                if i < NCH:
                    emit_sc(prev_tb, prev, i)
                if i >= 1:
                    emit_rest(prev_tb, prev, i - 1)

            blocks = {}
            def mkblk():
                return {"qt": [], "kt": [], "vv": [], "kk": [], "at": None,
                        "sT": {}}

            blocks[0] = mkblk()
            emit_dma_h(0, blocks[0])
            wdefer["wkh"] = load_w("wkh", WKTH)
            wdefer["wkm"] = load_w("wkm", WKTM)
            emit_dma_m(0, blocks[0])
            wdefer["wvh"] = load_w("wvh", WVTH)
            wdefer["wvm"] = load_w("wvm", WVTM)

            for tb in range(NTB + 1):
                cur = blocks.get(tb)
                prev = blocks.get(tb - 1)
                ptb = tb - 1
                if cur is not None:
                    if tb + 1 < NTB:
                        blocks[tb + 1] = mkblk()
                        emit_dma_h(tb + 1, blocks[tb + 1])
                    for hh in range(HPC):
                        emit_q(tb, cur, hh)
                        AU(ptb, prev, hh)
                    for hh in range(HPC):
                        emit_k(tb, cur, hh)
                        AU(ptb, prev, HPC + hh)
                    emit_v(tb, cur, 0)
                    AU(ptb, prev, 8)
                    for ts in range(1, HPC):
                        emit_v(tb, cur, ts)
                    if tb + 1 < NTB:
                        emit_dma_m(tb + 1, blocks[tb + 1])
                    if prev is not None:
                        if debug:
                            for hh in range(HPC):
                                nc.sync.dma_start(
                                    ATDBG[hh * 128:(hh + 1) * 128,
                                          ptb * TB:(ptb + 1) * TB],
                                    prev["at"][hh][:])
                        for ts in range(HPC):
                            for fb in range(4):
                                emit_out(ptb, prev, ts, fb)
                    for ts in range(HPC):
                        emit_ktr(tb, cur, ts)
                else:
                    for i in range(NCH + 1):
                        AU(ptb, prev, i)
                    if debug:
                        for hh in range(HPC):
                            nc.sync.dma_start(
                                ATDBG[hh * 128:(hh + 1) * 128,
                                      ptb * TB:(ptb + 1) * TB],
                                prev["at"][hh][:])
                    for ts in range(HPC):
                        for fb in range(4):
                            emit_out(ptb, prev, ts, fb)
                blocks.pop(tb - 2, None)

            stf = cst.tile([128, JW], f32, tag="stf")
            nc.vector.tensor_copy(stf[:], st_ps[:])
            nc.sync.dma_start(OST[:], stf[:])

    nc.compile()
    return nc


def _get_nc():
    global _cached_nc
    if _cached_nc is None:
        _cached_nc = _build()
    return _cached_nc


def _prep_inputs(x, Wq, Wk, Wv, Wo):
    bf = ml_dtypes.bfloat16
    f32 = np.float32
    s = 1.0 / math.sqrt(DH)

    def split_t(W):
        Wh = W.astype(bf)
        Wm = (W - Wh.astype(f32)).astype(bf)
        return (np.ascontiguousarray(Wh.T), np.ascontiguousarray(Wm.T))

    xts = []
    for b in range(B):
        xb = np.asarray(x[b], dtype=f32)
        xh = xb.astype(bf)
        xm = (xb - xh.astype(f32)).astype(bf)
        xts.append((np.ascontiguousarray(xh.T), np.ascontiguousarray(xm.T)))

    m1 = np.triu(np.ones((C, C), dtype=f32))
    mask = np.ascontiguousarray(np.tile(np.vstack([m1, m1]), (1, HPC)))
    idn = np.eye(128, dtype=f32).astype(bf)

    in_maps = []
    for c in range(NCORES):
        b, g = divmod(c, GPB)
        rows = slice(g * JW, (g + 1) * JW)
        wqth = np.ascontiguousarray(((Wq[rows] * s).astype(bf)).T)
        wkth, wktm = split_t(Wk[rows])
        wvth, wvtm = split_t(Wv[rows])
        wot = np.ascontiguousarray(Wo[:, rows].T.astype(bf))
        in_maps.append({
            "XTH": xts[b][0], "XTM": xts[b][1],
            "WQTH": wqth, "WKTH": wkth, "WKTM": wktm,
            "WVTH": wvth, "WVTM": wvtm, "WOT": wot,
            "MSK": mask, "IDN": idn,
        })
    return in_maps


def kernel(x, Wq, Wk, Wv, Wo, _trace=False, _trace_kwargs=None):
    x, Wq, Wk, Wv, Wo = (np.asarray(a, dtype=np.float32)
                         for a in (x, Wq, Wk, Wv, Wo))
    nc = _get_nc()
    in_maps = _prep_inputs(x, Wq, Wk, Wv, Wo)
    res = run_bass_kernel_spmd(nc, in_maps, list(range(NCORES)),
                               trace=_trace, **(_trace_kwargs or {}))
    out = np.zeros((B, T, D), np.float32)
    state = np.zeros((B, H, DH, DH), np.float32)
    for c in range(NCORES):
        b, g = divmod(c, GPB)
        out[b] += res.results[c]["OUT"]
        st = res.results[c]["OST"]
        for hh in range(HPC):
            state[b, g * HPC + hh] = st[:, hh * 128:(hh + 1) * 128]
    kernel.last_result = res
    return out, state


# revision 13
# speedup vs baseline: 1.0246x; 1.0103x over previous
"""Trainium2 Bass kernel for BinaryAssociativeMemory (chunked binary linear attention).

Sharding: 8 cores = 2 batches x 4 head-groups (4 heads each).
Per core: q/k/v projections from pre-transposed x (host supplies bf16 hi/mid
split), k/v binarized from a 3-term bf16 split matmul (sign-exact vs fp32),
chunked attention with a PSUM-resident running k^T v state, and a partial
output projection over the core's 4 heads. Host sums the 4 head-group
partials per batch.

The per-block schedule is software-pipelined: block tb's dense projection
matmuls are interleaved with block tb-1's attention chunks so the PE never
sees a sparse window (HAM stays un-throttled) and the attention's DVE
dependencies resolve during the projection units.
"""

import math
import numpy as np
import ml_dtypes

import concourse.bacc as bacc
import concourse.mybir as mybir
import concourse.tile as tile
from concourse.bass_utils import run_bass_kernel_spmd

dt = mybir.dt
AF = mybir.ActivationFunctionType

B, T, D = 2, 4096, 2048
H, DH, C = 16, 128, 64
NCORES = 8
GPB = 4            # head-groups per batch
HPC = H // GPB     # heads per core = 4
JW = HPC * DH      # per-core projection width = 512
TB = 512           # timestep block
NTB = T // TB      # 8
ND = D // 128      # 16 contraction blocks
NCH = TB // C      # 8 chunks per block
NCHUNK = T // C    # 64 chunks total

_cached_nc = None


def _build(debug=False):
    nc = bacc.Bacc("TRN2", target_bir_lowering=False, debug=False,
                   enable_asserts=False, num_devices=NCORES)
    f32, bf16 = dt.float32, dt.bfloat16

    XTH = nc.dram_tensor("XTH", [D, T], bf16, kind="ExternalInput").ap()
    XTM = nc.dram_tensor("XTM", [D, T], bf16, kind="ExternalInput").ap()
    WQTH = nc.dram_tensor("WQTH", [D, JW], bf16, kind="ExternalInput").ap()
    WKTH = nc.dram_tensor("WKTH", [D, JW], bf16, kind="ExternalInput").ap()
    WKTM = nc.dram_tensor("WKTM", [D, JW], bf16, kind="ExternalInput").ap()
    WVTH = nc.dram_tensor("WVTH", [D, JW], bf16, kind="ExternalInput").ap()
    WVTM = nc.dram_tensor("WVTM", [D, JW], bf16, kind="ExternalInput").ap()
    WOT = nc.dram_tensor("WOT", [JW, D], bf16, kind="ExternalInput").ap()
    MSK = nc.dram_tensor("MSK", [128, HPC * C], f32, kind="ExternalInput").ap()
    IDN = nc.dram_tensor("IDN", [128, 128], bf16, kind="ExternalInput").ap()
    OUT = nc.dram_tensor("OUT", [T, D], f32, kind="ExternalOutput").ap()
    OST = nc.dram_tensor("OST", [128, JW], f32, kind="ExternalOutput").ap()
    if debug:
        QDBG = nc.dram_tensor("QDBG", [JW, T], bf16, kind="ExternalOutput").ap()
        KDBG = nc.dram_tensor("KDBG", [JW, T], bf16, kind="ExternalOutput").ap()
        VDBG = nc.dram_tensor("VDBG", [T, JW], bf16, kind="ExternalOutput").ap()
        KKDBG = nc.dram_tensor("KKDBG", [T, JW], bf16, kind="ExternalOutput").ap()
        ATDBG = nc.dram_tensor("ATDBG", [JW, T], bf16, kind="ExternalOutput").ap()

    with tile.TileContext(nc) as tc:
        with (
            tc.tile_pool(name="const", bufs=1) as cst,
            tc.tile_pool(name="xblk", bufs=1) as xblk,
            tc.tile_pool(name="proj", bufs=8) as projp,
            tc.tile_pool(name="small", bufs=4) as smallp,
            tc.tile_pool(name="pfx", bufs=2) as pfxp,
            tc.tile_pool(name="bigps", bufs=4, space="PSUM") as bigps,
            tc.tile_pool(name="stps", bufs=1, space="PSUM") as stps,
            tc.tile_pool(name="scat", bufs=3, space="PSUM") as scat,
        ):
            # ---- constants / weights resident in SBUF ----
            def load_w(name, dram):
                t = cst.tile([128, ND * JW], bf16, tag=name, name=name)
                for kb in range(ND):
                    nc.sync.dma_start(t[:, kb * JW:(kb + 1) * JW],
                                      dram[kb * 128:(kb + 1) * 128, :])
                return t

            wq = load_w("wq", WQTH)
            wo = cst.tile([128, HPC * D], bf16, tag="wo")
            for hh in range(HPC):
                nc.sync.dma_start(wo[:, hh * D:(hh + 1) * D],
                                  WOT[hh * 128:(hh + 1) * 128, :])
            msk = cst.tile([128, HPC * C], f32, tag="msk")
            nc.sync.dma_start(msk[:], MSK[:])
            idn = cst.tile([128, 128], bf16, tag="idn")
            nc.sync.dma_start(idn[:], IDN[:])

            st_ps = stps.tile([128, JW], f32, tag="st")
            pipe = {"pf": None}
            wdefer = {}

            # ---------- emission units ----------
            def emit_dma_h(tb, blk):
                t0 = tb * TB
                xh = xblk.tile([128, ND * TB], bf16, tag="xh", name=f"xh{tb}", bufs=2)
                for kb in range(ND):
                    nc.sync.dma_start(xh[:, kb * TB:(kb + 1) * TB],
                                      XTH[kb * 128:(kb + 1) * 128, t0:t0 + TB])
                blk["xh"] = xh

            def emit_dma_m(tb, blk):
                t0 = tb * TB
                xm = xblk.tile([128, ND * TB], bf16, tag="xm", name=f"xm{tb}", bufs=1)
                for kb in range(ND):
                    nc.sync.dma_start(xm[:, kb * TB:(kb + 1) * TB],
                                      XTM[kb * 128:(kb + 1) * 128, t0:t0 + TB])
                blk["xm"] = xm

            def emit_q(tb, blk, hh):
                ps = bigps.tile([128, TB], f32, tag="bigps", name=f"psq{tb}_{hh}")
                for kb in range(ND):
                    nc.tensor.matmul(
                        ps[:], wq[:, kb * JW + hh * 128: kb * JW + (hh + 1) * 128],
                        blk["xh"][:, kb * TB:(kb + 1) * TB],
                        start=(kb == 0), stop=(kb == ND - 1))
                q_h = projp.tile([128, TB], bf16, tag="qt", name=f"qt{tb}_{hh}")
                nc.vector.tensor_copy(q_h[:], ps[:])
                if debug:
                    t0 = tb * TB
                    nc.sync.dma_start(QDBG[hh * 128:(hh + 1) * 128, t0:t0 + TB], q_h[:])
                blk["qt"].append(q_h)

            def emit_k(tb, blk, hh):
                ps = bigps.tile([128, TB], f32, tag="bigps", name=f"psk{tb}_{hh}")
                i, n_mm = 0, 3 * ND
                for kb in range(ND):
                    for w_t, x_t in ((wdefer["wkh"], blk["xh"]), (wdefer["wkh"], blk["xm"]), (wdefer["wkm"], blk["xh"])):
                        nc.tensor.matmul(
                            ps[:], w_t[:, kb * JW + hh * 128: kb * JW + (hh + 1) * 128],
                            x_t[:, kb * TB:(kb + 1) * TB],
                            start=(i == 0), stop=(i == n_mm - 1))
                        i += 1
                k_h = projp.tile([128, TB], bf16, tag="kt", name=f"kt{tb}_{hh}")
                nc.scalar.activation(k_h[:], ps[:], AF.Sign)
                if debug:
                    t0 = tb * TB
                    nc.sync.dma_start(KDBG[hh * 128:(hh + 1) * 128, t0:t0 + TB], k_h[:])
                blk["kt"].append(k_h)

            def emit_v(tb, blk, ts):
                ps = bigps.tile([128, JW], f32, tag="bigps", name=f"psv{tb}_{ts}")
                i, n_mm = 0, 3 * ND
                for kb in range(ND):
                    for x_t, w_t in ((blk["xh"], wdefer["wvh"]), (blk["xh"], wdefer["wvm"]), (blk["xm"], wdefer["wvh"])):
                        nc.tensor.matmul(
                            ps[:], x_t[:, kb * TB + ts * 128: kb * TB + (ts + 1) * 128],
                            w_t[:, kb * JW:(kb + 1) * JW],
                            start=(i == 0), stop=(i == n_mm - 1))
                        i += 1
                v_ts = projp.tile([128, JW], bf16, tag="vv", name=f"vv{tb}_{ts}")
                nc.scalar.activation(v_ts[:], ps[:], AF.Sign)
                if debug:
                    t0 = tb * TB
                    nc.sync.dma_start(VDBG[t0 + ts * 128: t0 + (ts + 1) * 128, :], v_ts[:])
                blk["vv"].append(v_ts)

            def emit_ktr(tb, blk, ts):
                k_ts = projp.tile([128, JW], bf16, tag="kk", name=f"kk{tb}_{ts}")
                for hh in range(HPC):
                    tp = scat.tile([128, 128], bf16, tag="scat", name=f"tp{tb}_{ts}_{hh}")
                    nc.tensor.transpose(tp[:], blk["kt"][hh][:, ts * 128:(ts + 1) * 128],
                                        idn[:])
                    nc.vector.tensor_copy(k_ts[:, hh * 128:(hh + 1) * 128], tp[:])
                if debug:
                    t0 = tb * TB
                    nc.sync.dma_start(KKDBG[t0 + ts * 128: t0 + (ts + 1) * 128, :], k_ts[:])
                blk["kk"].append(k_ts)

            def emit_sc(tb, blk, ch):
                # scores for all 4 heads of one chunk -> one PSUM bank; mask in 1 DVE op
                ro = (ch % 2) * C
                c0 = ch * C
                sc = scat.tile([128, HPC * C], f32, tag="scat", name=f"sc{tb}_{ch}")
                for hh in range(HPC):
                    nc.tensor.matmul(sc[ro:ro + C, hh * C:(hh + 1) * C],
                                     blk["kt"][hh][:, c0:c0 + C],
                                     blk["qt"][hh][:, c0:c0 + C],
                                     start=(hh == 0), stop=True,
                                     skip_group_check=True)
                sT = smallp.tile([128, HPC * C], bf16, tag="sT", name=f"sT{tb}_{ch}")
                nc.vector.tensor_mul(sT[ro:ro + C, :], sc[ro:ro + C, :], msk[ro:ro + C, :])
                blk["sT"][ch] = sT

            def emit_rest(tb, blk, ch):
                gn = tb * NCH + ch
                ts, ro = ch // 2, (ch % 2) * C
                c0 = ch * C
                if not blk["at"]:
                    blk["at"] = [projp.tile([128, TB], bf16, tag="at", bufs=5,
                                            name=f"at{tb}_{hh}") for hh in range(HPC)]
                sT = blk["sT"].pop(ch)
                ap = scat.tile([128, HPC * C], f32, tag="scat", name=f"ap{tb}_{ch}")
                for hh in range(HPC):
                    nc.tensor.matmul(ap[:, hh * C:(hh + 1) * C],
                                     blk["vv"][ts][ro:ro + C, hh * 128:(hh + 1) * 128],
                                     sT[ro:ro + C, hh * C:(hh + 1) * C],
                                     start=(hh == 0), stop=(gn == 0),
                                     skip_group_check=True)
                if gn > 0:
                    for hh in range(HPC):
                        nc.tensor.matmul(ap[:, hh * C:(hh + 1) * C],
                                         pipe["pf"][:, hh * 128:(hh + 1) * 128],
                                         blk["qt"][hh][:, c0:c0 + C],
                                         start=False, stop=True,
                                         skip_group_check=True)
                for hh in range(HPC):
                    nc.vector.tensor_copy(blk["at"][hh][:, c0:c0 + C],
                                          ap[:, hh * C:(hh + 1) * C])
                for hh in range(HPC):
                    # start=True clears has_written for the WHOLE bank: only the
                    # very first write of the kernel-long group may set it.
                    nc.tensor.matmul(st_ps[:, hh * 128:(hh + 1) * 128],
                                     blk["kk"][ts][ro:ro + C, hh * 128:(hh + 1) * 128],
                                     blk["vv"][ts][ro:ro + C, hh * 128:(hh + 1) * 128],
                                     start=(gn == 0 and hh == 0),
                                     stop=(gn == NCHUNK - 1),
                                     skip_group_check=True)
                if gn < NCHUNK - 1:
                    pf_new = pfxp.tile([128, JW], bf16, tag="pf", name=f"pf{gn}")
                    nc.vector.tensor_copy(pf_new[:], st_ps[:])
                    pipe["pf"] = pf_new

            def emit_out(tb, blk, ts, fb):
                t0 = tb * TB
                op = bigps.tile([128, 512], f32, tag="bigps", name=f"po{tb}_{ts}_{fb}")
                for hh in range(HPC):
                    nc.tensor.matmul(
                        op[:], blk["at"][hh][:, ts * 128:(ts + 1) * 128],
                        wo[:, hh * D + fb * 512: hh * D + (fb + 1) * 512],
                        start=(hh == 0), stop=(hh == HPC - 1))
                ob = smallp.tile([128, 512], f32, tag="ob", bufs=3, name=f"ob{tb}_{ts}_{fb}")
                nc.any.tensor_copy(ob[:], op[:])
                nc.sync.dma_start(
                    OUT[t0 + ts * 128: t0 + (ts + 1) * 128,
                        fb * 512:(fb + 1) * 512], ob[:])

            # ---------- pipelined schedule ----------
            def AU(prev_tb, prev, i):
                """attention micro-unit i (0..8) for the previous block"""
                if prev is None:
                    return
                if i < NCH:
                    emit_sc(prev_tb, prev, i)
                if i >= 1:
                    emit_rest(prev_tb, prev, i - 1)

            blocks = {}
            def mkblk():
                return {"qt": [], "kt": [], "vv": [], "kk": [], "at": None,
                        "sT": {}}

            blocks[0] = mkblk()
            emit_dma_h(0, blocks[0])
            wdefer["wkh"] = load_w("wkh", WKTH)
            wdefer["wkm"] = load_w("wkm", WKTM)
            emit_dma_m(0, blocks[0])
            wdefer["wvh"] = load_w("wvh", WVTH)
            wdefer["wvm"] = load_w("wvm", WVTM)

            for tb in range(NTB + 1):
                cur = blocks.get(tb)
                prev = blocks.get(tb - 1)
                ptb = tb - 1
                if cur is not None:
                    if tb + 1 < NTB:
                        blocks[tb + 1] = mkblk()
                        emit_dma_h(tb + 1, blocks[tb + 1])
                    for hh in range(HPC):
                        emit_q(tb, cur, hh)
                        AU(ptb, prev, hh)
                    for hh in range(HPC):
                        emit_k(tb, cur, hh)
                        AU(ptb, prev, HPC + hh)
                    emit_v(tb, cur, 0)
                    AU(ptb, prev, 8)
                    for ts in range(1, HPC):
                        emit_v(tb, cur, ts)
                    if tb + 1 < NTB:
                        emit_dma_m(tb + 1, blocks[tb + 1])
                    if prev is not None:
                        if debug:
                            for hh in range(HPC):
                                nc.sync.dma_start(
                                    ATDBG[hh * 128:(hh + 1) * 128,
                                          ptb * TB:(ptb + 1) * TB],
                                    prev["at"][hh][:])
                        for ts in range(HPC):
                            for fb in range(4):
                                emit_out(ptb, prev, ts, fb)
                    for ts in range(HPC):
                        emit_ktr(tb, cur, ts)
                else:
                    for i in range(NCH + 1):
                        AU(ptb, prev, i)
                        # after AU(i), chunks 0..i-1 are complete; out t-sub ts
                        # needs chunks 2ts,2ts+1 -> ready after AU(2ts+2).
                        if i >= 2 and i % 2 == 0:
                            for fb in range(4):
                                emit_out(ptb, prev, (i - 2) // 2, fb)
                    if debug:
                        for hh in range(HPC):
                            nc.sync.dma_start(
                                ATDBG[hh * 128:(hh + 1) * 128,
                                      ptb * TB:(ptb + 1) * TB],
                                prev["at"][hh][:])
                blocks.pop(tb - 2, None)

            stf = cst.tile([128, JW], f32, tag="stf")
            nc.vector.tensor_copy(stf[:], st_ps[:])
            nc.sync.dma_start(OST[:], stf[:])

    nc.compile()
    return nc


def _get_nc():
    global _cached_nc
    if _cached_nc is None:
        _cached_nc = _build()
    return _cached_nc


def _prep_inputs(x, Wq, Wk, Wv, Wo):
    bf = ml_dtypes.bfloat16
    f32 = np.float32
    s = 1.0 / math.sqrt(DH)

    def split_t(W):
        Wh = W.astype(bf)
        Wm = (W - Wh.astype(f32)).astype(bf)
        return (np.ascontiguousarray(Wh.T), np.ascontiguousarray(Wm.T))

    xts = []
    for b in range(B):
        xb = np.asarray(x[b], dtype=f32)
        xh = xb.astype(bf)
        xm = (xb - xh.astype(f32)).astype(bf)
        xts.append((np.ascontiguousarray(xh.T), np.ascontiguousarray(xm.T)))

    m1 = np.triu(np.ones((C, C), dtype=f32))
    mask = np.ascontiguousarray(np.tile(np.vstack([m1, m1]), (1, HPC)))
    idn = np.eye(128, dtype=f32).astype(bf)

    in_maps = []
    for c in range(NCORES):
        b, g = divmod(c, GPB)
        rows = slice(g * JW, (g + 1) * JW)
        wqth = np.ascontiguousarray(((Wq[rows] * s).astype(bf)).T)
        wkth, wktm = split_t(Wk[rows])
        wvth, wvtm = split_t(Wv[rows])
        wot = np.ascontiguousarray(Wo[:, rows].T.astype(bf))
        in_maps.append({
            "XTH": xts[b][0], "XTM": xts[b][1],
            "WQTH": wqth, "WKTH": wkth, "WKTM": wktm,
            "WVTH": wvth, "WVTM": wvtm, "WOT": wot,
            "MSK": mask, "IDN": idn,
        })
    return in_maps


def kernel(x, Wq, Wk, Wv, Wo, _trace=False, _trace_kwargs=None):
    x, Wq, Wk, Wv, Wo = (np.asarray(a, dtype=np.float32)
                         for a in (x, Wq, Wk, Wv, Wo))
    nc = _get_nc()
    in_maps = _prep_inputs(x, Wq, Wk, Wv, Wo)
    res = run_bass_kernel_spmd(nc, in_maps, list(range(NCORES)),
                               trace=_trace, **(_trace_kwargs or {}))
    out = np.zeros((B, T, D), np.float32)
    state = np.zeros((B, H, DH, DH), np.float32)
    for c in range(NCORES):
        b, g = divmod(c, GPB)
        out[b] += res.results[c]["OUT"]
        st = res.results[c]["OST"]
        for hh in range(HPC):
            state[b, g * HPC + hh] = st[:, hh * 128:(hh + 1) * 128]
    kernel.last_result = res
    return out, state
